# revision 1
# baseline (speedup 1.0000x reference)
"""Block-sparse self-attention (block=20, heads=4) on 8 TRN2 NeuronCores.

Strategy: data-parallel over batch B=32 -> 4 sequences per core; weights
replicated. Fully fused on-chip pipeline per 480-token chunk (no HBM
round-trips for qkv/attention intermediates):

  x^T (host-pretransposed, bf16)  --DMA-->  x_fm [d,t] SBUF
  q,k: feature-major projections (PE, lhsT=W^T chunks, rhs=x_fm)
  v:   token-major projection    (PE, lhsT=x_fm, rhs=Wv^T)
  per 120-token subtile (6 blocks):
    S = (q*s)^T k per head, 2-head row-packed matmuls -> PSUM
    exp on ACT (no max-subtraction: |S| <~ 20 so exp is safe in fp32/bf16)
    block-diag 0/1 mask multiply, row-sum, reciprocal, scale  (DVE)
    A^T via PE transpose; AV col-packed -> o feature-major
    out-proj: lhsT=o_fm, rhs=Wo^T -> y token-major PSUM -> SBUF -> DMA out

All matmuls bf16 inputs with fp32 PSUM accumulation. Biases are applied
generally (ACT per-partition bias for q/k; rank-1 ones-matmul for v and y).
"""

import numpy as np
import ml_dtypes

import concourse.bass as bass
import concourse.mybir as mybir
import concourse.tile as tile
from concourse import bacc
from concourse.bass_utils import run_bass_kernel_spmd

F32 = mybir.dt.float32
BF16 = mybir.dt.bfloat16

B, T, D = 32, 4000, 256
BS = 20            # attention block size
H = 4              # heads
HD = D // H        # 64
NCORES = 8
BPC = B // NCORES  # sequences per core

CHUNK = 480        # tokens per chunk (24 blocks)
SUB = 120          # tokens per subtile (6 blocks), M-dim of attention matmuls


def _chunks_for(t_total):
    """Yield (t0, [subtile sizes]) covering t_total tokens."""
    out = []
    t0 = 0
    while t0 < t_total:
        ch = min(CHUNK, t_total - t0)
        subs = []
        off = 0
        while off < ch:
            subs.append(min(SUB, ch - off))
            off += subs[-1]
        out.append((t0, subs))
        t0 += ch
    return out


def build_program(bpc=BPC, t_total=T, vy_bias=True):
    nc = bacc.Bacc("TRN2", target_bir_lowering=False, debug=False,
                   num_devices=NCORES)

    # ---- DRAM I/O ----
    xT = nc.dram_tensor("xT", [bpc, D, t_total], BF16, kind="ExternalInput")
    wqkT = nc.dram_tensor("wqkT", [D, 2 * D], BF16, kind="ExternalInput")
    wvT = nc.dram_tensor("wvT", [D, D], BF16, kind="ExternalInput")
    woT = nc.dram_tensor("woT", [D, D], BF16, kind="ExternalInput")
    bqk = nc.dram_tensor("bqk", [4, 128], F32, kind="ExternalInput")
    bv = nc.dram_tensor("bv", [1, D], BF16, kind="ExternalInput")
    by = nc.dram_tensor("by", [1, D], BF16, kind="ExternalInput")
    mU = nc.dram_tensor("mU", [7, SUB], BF16, kind="ExternalInput")
    mW = nc.dram_tensor("mW", [7, 2 * SUB], BF16, kind="ExternalInput")
    onesc = nc.dram_tensor("onesc", [1, SUB], BF16, kind="ExternalInput")
    identc = nc.dram_tensor("identc", [128, 128], BF16, kind="ExternalInput")
    y = nc.dram_tensor("y", [bpc, t_total, D], F32, kind="ExternalOutput")

    xT_r = xT.rearrange("b (dk p) t -> b p dk t", p=128)
    wqkT_r = wqkT.rearrange("(dk p) e -> p dk e", p=128)
    wvT_r = wvT.rearrange("(dk p) c -> p dk c", p=128)
    woT_r = woT.rearrange("(ek p) c -> p ek c", p=128)
    bqk_r = bqk.rearrange("c p -> p c")

    with tile.TileContext(nc) as tc:
        with (
            tc.tile_pool(name="consts", bufs=1) as cpool,
            tc.tile_pool(name="xf", bufs=3) as xpool,
            tc.tile_pool(name="qk", bufs=3) as qkpool,
            tc.tile_pool(name="att", bufs=4) as apool,
            tc.tile_pool(name="out", bufs=4) as opool,
            tc.tile_pool(name="ps", bufs=8, space="PSUM") as pspool,
        ):
            # ---- constants in SBUF ----
            wqk_sb = cpool.tile([128, 2, 2 * D], BF16, tag="wqk")
            nc.sync.dma_start(out=wqk_sb, in_=wqkT_r)
            wv_sb = cpool.tile([128, 2, D], BF16, tag="wv")
            nc.sync.dma_start(out=wv_sb, in_=wvT_r)
            wo_sb = cpool.tile([128, 2, D], BF16, tag="wo")
            nc.sync.dma_start(out=wo_sb, in_=woT_r)
            bqk_sb = cpool.tile([128, 4], F32, tag="bqk")
            nc.sync.dma_start(out=bqk_sb, in_=bqk_r)
            bv_sb = cpool.tile([1, D], BF16, tag="bv")
            nc.sync.dma_start(out=bv_sb, in_=bv[:, :])
            by_sb = cpool.tile([1, D], BF16, tag="by")
            nc.sync.dma_start(out=by_sb, in_=by[:, :])
            mU_sb = cpool.tile([7, SUB], BF16, tag="mU")
            nc.sync.dma_start(out=mU_sb, in_=mU[:, :])
            mW_sb = cpool.tile([7, 2 * SUB], BF16, tag="mW")
            nc.sync.dma_start(out=mW_sb, in_=mW[:, :])
            ones_sb = cpool.tile([1, SUB], BF16, tag="ones")
            nc.sync.dma_start(out=ones_sb, in_=onesc[:, :])
            id_sb = cpool.tile([128, 128], BF16, tag="ident")
            nc.sync.dma_start(out=id_sb, in_=identc[:, :])

            for b in range(bpc):
                for (t0, subs) in _chunks_for(t_total):
                    ch = sum(subs)
                    # ---- load x^T chunk: [128, 2, ch] bf16 ----
                    xfm = xpool.tile([128, 2, CHUNK], BF16, tag="xfm")
                    nc.sync.dma_start(out=xfm[:, :, :ch],
                                      in_=xT_r[b, :, :, t0:t0 + ch])

                    # ---- q, k feature-major projections ----
                    # pc 0,1 = q e-chunks; 2,3 = k e-chunks
                    qk_sb = []
                    for pc in range(4):
                        ps = pspool.tile([128, CHUNK], F32, tag="ps")
                        for dk in range(2):
                            nc.tensor.matmul(
                                ps[:, :ch],
                                wqk_sb[:, dk, pc * 128:(pc + 1) * 128],
                                xfm[:, dk, :ch],
                                start=(dk == 0), stop=(dk == 1),
                            )
                        sb = qkpool.tile([128, CHUNK], BF16, tag=f"qk{pc}")
                        scale = 0.125 if pc < 2 else 1.0
                        nc.scalar.activation(
                            sb[:, :ch], ps[:, :ch],
                            mybir.ActivationFunctionType.Identity,
                            bias=bqk_sb[:, pc:pc + 1], scale=scale,
                        )
                        qk_sb.append(sb)

                    # ---- v token-major projection (per subtile) ----
                    vtm = xpool.tile([SUB, len(subs), D], BF16, tag="vtm")
                    off = 0
                    for si, s in enumerate(subs):
                        vps = pspool.tile([SUB, D], F32, tag="ps")
                        for dk in range(2):
                            nc.tensor.matmul(
                                vps[:s, :],
                                xfm[:, dk, off:off + s],
                                wv_sb[:, dk, :],
                                start=(dk == 0),
                                stop=(dk == 1 and not vy_bias),
                            )
                        if vy_bias:
                            nc.tensor.matmul(
                                vps[:s, :], ones_sb[:, :s], bv_sb[:, :],
                                start=False, stop=True,
                            )
                        nc.vector.tensor_copy(vtm[:s, si, :], vps[:s, :])
                        off += s

                    # ---- attention + out-proj, stage-major across subtiles ----
                    offs = []
                    o = 0
                    for sz in subs:
                        offs.append(o)
                        o += sz
                    nsub = len(subs)

                    # stage 1: scores, 2 banks (A: h0,h2 @row0; B: h1,h3 @row64)
                    sps_l = []
                    for si, s in enumerate(subs):
                        tw = slice(offs[si], offs[si] + s)
                        sA = pspool.tile([SUB, 2, SUB], F32, tag="ps")
                        sB = pspool.tile([SUB, 2, SUB], F32, tag="ps")
                        for h in range(H):
                            rp = (h % 2) * 64
                            dst = sA if h % 2 == 0 else sB
                            nc.tensor.matmul(
                                dst[:s, h // 2, :s],
                                qk_sb[h // 2][rp:rp + 64, tw],
                                qk_sb[2 + h // 2][rp:rp + 64, tw],
                                start=(h < 2), stop=False,
                                tile_position=(rp, 0),
                            )
                        # accumulate the -64 off-block additive mask (rank-7)
                        for dst in (sA, sB):
                            mwv = mW_sb.rearrange("p (a b) -> p a b", a=2)
                            nc.tensor.matmul(
                                dst[:s, :, :s], mU_sb[:, :s],
                                mwv[:, :, :s],
                                start=False, stop=True,
                            )
                        sps_l.append((sA, sB))

                    # stage 2: exp per score bank -> ee slots (h0,h2,h1,h3),
                    # then fused mask-multiply + row-sum (DVE TTR) per head.
                    # slot(h) = 2*(h%2) + h//2
                    ee_l = []
                    den_l = []
                    for si, s in enumerate(subs):
                        sA, sB = sps_l[si]
                        ee = apool.tile([SUB, 4, SUB], BF16, tag="ee")
                        den = apool.tile([SUB, 4], F32, tag="den")
                        nc.scalar.activation(ee[:s, 0:2, :s], sA[:s, :, :s],
                                             mybir.ActivationFunctionType.Exp)
                        nc.scalar.activation(ee[:s, 2:4, :s], sB[:s, :, :s],
                                             mybir.ActivationFunctionType.Exp)
                        nc.vector.reduce_sum(den[:s, :], ee[:s, :, :s],
                                             axis=mybir.AxisListType.X)
                        ee_l.append(ee)
                        den_l.append(den)

                    # stage 3: reciprocal + scale (DVE)
                    for si, s in enumerate(subs):
                        ee = ee_l[si]
                        rec = apool.tile([SUB, 4], F32, tag="rec")
                        nc.vector.reciprocal(rec[:s, :], den_l[si][:s, :])
                        rec_b = bass.AP(
                            tensor=rec.tensor, offset=rec.offset,
                            ap=[rec.ap[0][:], [rec.ap[1][0], 4], [0, s]],
                        )[:s]
                        nc.vector.tensor_mul(ee[:s, :, :s], ee[:s, :, :s],
                                             rec_b)

                    # stage 4: A^T via PE transpose + copy to SBUF
                    at_l = []
                    for si, s in enumerate(subs):
                        atps = pspool.tile([SUB, 4, SUB], BF16, tag="ps")
                        for h in range(H):
                            sl = 2 * (h % 2) + h // 2
                            nc.tensor.transpose(atps[:s, h, :s],
                                                ee_l[si][:s, sl, :s],
                                                id_sb[:s, :s])
                        at_sb = apool.tile([SUB, 4, SUB], BF16, tag="at")
                        nc.vector.tensor_copy(at_sb[:s, :, :s], atps[:s, :, :s])
                        at_l.append(at_sb)

                    # stage 5: AV col-packed pairs -> o feature-major
                    o_l = []
                    for si, s in enumerate(subs):
                        o_sb = opool.tile([128, 2, SUB], BF16, tag="osb")
                        for pair in range(2):
                            ops = pspool.tile([128, SUB], F32, tag="ps")
                            for hh in range(2):
                                h = pair * 2 + hh
                                cp = hh * 64
                                nc.tensor.matmul(
                                    ops[cp:cp + 64, :s],
                                    vtm[:s, si, h * HD:(h + 1) * HD],
                                    at_l[si][:s, h, :s],
                                    start=True, stop=True,
                                    tile_position=(0, cp),
                                )
                            nc.vector.tensor_copy(o_sb[:, pair, :s],
                                                  ops[:, :s])
                        o_l.append(o_sb)

                    # stage 6: out-proj -> y token-major -> DMA
                    for si, s in enumerate(subs):
                        yps = pspool.tile([SUB, D], F32, tag="ps")
                        for ec in range(2):
                            nc.tensor.matmul(
                                yps[:s, :],
                                o_l[si][:, ec, :s],
                                wo_sb[:, ec, :],
                                start=(ec == 0),
                                stop=(ec == 1 and not vy_bias),
                            )
                        if vy_bias:
                            nc.tensor.matmul(
                                yps[:s, :], ones_sb[:, :s], by_sb[:, :],
                                start=False, stop=True,
                            )
                        y_sb = opool.tile([SUB, D], F32, tag="ysb")
                        nc.vector.tensor_copy(y_sb[:s, :], yps[:s, :])
                        t0s = t0 + offs[si]
                        nc.sync.dma_start(out=y[b, t0s:t0s + s, :],
                                          in_=y_sb[:s, :])

    nc.compile()
    return nc


_PROG = {}


def _get_program(bpc, t_total, vy_bias=True):
    key = (bpc, t_total, vy_bias)
    if key not in _PROG:
        _PROG[key] = build_program(bpc, t_total, vy_bias)
    return _PROG[key]


def _bf(a):
    return np.ascontiguousarray(a.astype(ml_dtypes.bfloat16))


def kernel(x, in_proj_w, in_proj_b, out_proj_w, out_proj_b):
    x = np.asarray(x, dtype=np.float32)
    in_proj_w = np.asarray(in_proj_w, dtype=np.float32)
    in_proj_b = np.asarray(in_proj_b, dtype=np.float32)
    out_proj_w = np.asarray(out_proj_w, dtype=np.float32)
    out_proj_b = np.asarray(out_proj_b, dtype=np.float32)

    b_total, t_total, d = x.shape
    bpc = b_total // NCORES
    vy_bias = bool(np.any(in_proj_b[2 * D:]) or np.any(out_proj_b))
    nc = _get_program(bpc, t_total, vy_bias)

    # host-side prep (shared weights)
    wqkT = _bf(in_proj_w[:2 * D].T)                      # [D, 512]
    wvT = _bf(in_proj_w[2 * D:].T)                       # [D, 256]
    woT = _bf(out_proj_w.T)                              # [D, 256]
    bqk = np.ascontiguousarray(
        in_proj_b[:2 * D].reshape(4, 128).astype(np.float32))
    bqk[:2] *= 0.125                                     # q bias pre-scaled
    bv = _bf(in_proj_b[2 * D:].reshape(1, D))
    by = _bf(out_proj_b.reshape(1, D))
    blk = np.arange(SUB) // BS
    mask1 = (blk[:, None] == blk[None, :]).astype(np.float32)
    C = 64.0
    mUv = np.zeros((7, SUB), np.float32)
    mWv = np.zeros((7, SUB), np.float32)
    mUv[0] = 1.0
    mWv[0] = -C
    for bb in range(SUB // BS):
        mUv[1 + bb, bb * BS:(bb + 1) * BS] = 8.0   # sqrt(64), exact in bf16
        mWv[1 + bb, bb * BS:(bb + 1) * BS] = 8.0
    mU_np = _bf(mUv)
    mW_np = _bf(np.concatenate([mWv, mWv], axis=1))  # [7, 2*SUB]
    onesc = _bf(np.ones((1, SUB), np.float32))
    identc = _bf(np.eye(128, dtype=np.float32))

    in_maps = []
    for c in range(NCORES):
        xs = x[c * bpc:(c + 1) * bpc]                    # [bpc, T, D]
        xT = _bf(xs.transpose(0, 2, 1))                  # [bpc, D, T]
        in_maps.append({
            "xT": xT, "wqkT": wqkT, "wvT": wvT, "woT": woT,
            "bqk": bqk, "bv": bv, "by": by,
            "mU": mU_np, "mW": mW_np, "onesc": onesc, "identc": identc,
        })

    global _last_in_maps
    _last_in_maps = in_maps
    res = run_bass_kernel_spmd(nc, in_maps, core_ids=list(range(NCORES)))
    out = np.concatenate([res.results[c]["y"] for c in range(NCORES)], axis=0)
    return out.astype(np.float32)


_last_in_maps = None

